# revision 28
# baseline (speedup 1.0000x reference)
"""Trainium2 Bass kernel for GQA attention (B=4, S=2048, HID=896, H=14, KV=2, D=64).

Sharding: 8 cores = 4 batches x 2 KV-head groups. Core c handles batch c//2,
query heads [g*7, (g+1)*7) with g = c%2 (exactly one KV head per core thanks to
GQA group structure). Each core computes its 448-channel slice of attn output
and the partial output projection y_g = ao_g @ Wo[g*448:(g+1)*448, :]; the host
sums the two partials per batch.

Per-core pipeline, interleaved in 4 super-blocks of 4 s-tiles each so the PE
never idles long enough for the HAM clock gate to re-throttle to 4/8:

  block i:
    A)  s-tile projections q = x@Wq_g (448 ch), kv = x@[Wk_g|Wv_g] (128 ch),
        emitted on a rolling schedule TWO tiles ahead of the block boundary so
        the PE has independent matmul work while the DVE RoPE/staging chain
        for the block drains. RoPE on q in natural [s, ch] layout on DVE; k
        RoPE'd in one batched DVE pass per block; PE-transposes to
        channel-major (q_pair head-pair tensors, kT2 duplicated into both
        partition halves).
    B)  q-chunk qc=i for ALL 7 heads: scoresT[k,q] = kT.T @ qT, structural
        causality (k-tiles <= diagonal; diagonal tiles exp'd then masked with
        a triangular 0/1 multiply). k-tile pairs share one [128,1024] PSUM +
        one exp; LOOKP pairs in flight. attn@[v|1] accumulates into PSUM
        [65,512]; row 64 = softmax denominators. Evacuation is per chunk: raw
        attn rows staged to SBUF, DVE fast-reciprocal of the den row, DRAM
        partition-broadcast roundtrip, one DVE multiply into aoT.
    C)  y s-tiles of block i-1 (emitted after B of block i, so the per-chunk
        normalization DMA chains have long resolved): y = ao @ Wo_g,
        PSUM -> SBUF -> DRAM. C(3) runs at the very end, shielded by C(2).

The causal mask input is never loaded: exp(-1e9 + s) == 0.0 exactly in fp32,
so structural masking matches the reference's additive mask bit-for-bit.

mm_dt selects the matmul dtype: float32r (2 PE cycles/row, ~1.4e-4 rounding =
RNE to 11 mantissa bits, replicated host-side) or float16 (1 cycle/row, ~2x
the rounding error) or float32 (exact, 4 cycles/row).
"""
import math
import os
import numpy as np

import concourse.bass as bass
import concourse.mybir as mybir
import concourse.tile as tile
from concourse import bacc
from concourse.masks import make_identity

F32 = mybir.dt.float32
F32R = mybir.dt.float32r
F16 = mybir.dt.float16
AF = mybir.ActivationFunctionType

B, S, HID = 4, 2048, 896
H, KV, D = 14, 2, 64
HL = H // KV          # 7 local query heads per core
GD = HL * D           # 448 local channels
KCH = HID // 128      # 7 contraction chunks
N_CORES = 8


def _bc7(ap_small):
    """[128, 32] cos/sin slice -> broadcast over the 7 heads: [128, 7, 32]."""
    return bass.AP(
        tensor=ap_small.tensor,
        offset=ap_small.offset,
        ap=[list(ap_small.ap[0]), [0, HL], list(ap_small.ap[1])],
    )


def build(s=S, mm_dt=None, reps=1, phases="A2BC"):
    if mm_dt is None:
        mm_dt = MM_DT
    ST = s // 128           # s-tiles
    QC = s // 512           # q chunks (also the number of super-blocks)
    TPB = ST // QC          # s-tiles per super-block (4)
    nc = bacc.Bacc("TRN2", target_bir_lowering=False, debug=False,
                   num_devices=N_CORES)

    xT = nc.dram_tensor("xT", [HID, s], mm_dt, kind="ExternalInput").ap()
    wq = nc.dram_tensor("wq", [HID, GD], mm_dt, kind="ExternalInput").ap()
    wkv = nc.dram_tensor("wkv", [HID, 128], mm_dt, kind="ExternalInput").ap()
    wo = nc.dram_tensor("wo", [GD, HID], mm_dt, kind="ExternalInput").ap()
    cosr = nc.dram_tensor("cosr", [128, ST, 32], F32, kind="ExternalInput").ap()
    sinr = nc.dram_tensor("sinr", [128, ST, 32], F32, kind="ExternalInput").ap()
    triu = nc.dram_tensor("triu", [128, 128], mm_dt, kind="ExternalInput").ap()
    y = nc.dram_tensor("y", [s, HID], F32, kind="ExternalOutput").ap()
    dram_rec = nc.dram_tensor("dram_rec", [HL, s], F32).ap()

    with tile.TileContext(nc) as tc:
        with (
            tc.tile_pool(name="wp", bufs=1) as wp,
            tc.tile_pool(name="per", bufs=1) as per,
            tc.tile_pool(name="tmp", bufs=2) as tmp,
            tc.tile_pool(name="expp", bufs=1) as expp,
            tc.tile_pool(name="rb", bufs=1) as rb,
            tc.tile_pool(name="xp", bufs=1) as xp,
        ):
            # ---- rep-invariant loads + persistent intermediates, hoisted out
            # of the rep loop: weights/tables/x stay resident in SBUF, so the
            # steady-state rep starts computing immediately ----
            wq_sb = wp.tile([128, KCH, GD], mm_dt, tag="wq", name="wq")
            nc.sync.dma_start(out=wq_sb[:], in_=wq.rearrange("(k p) m -> p k m", p=128))
            wkv_sb = wp.tile([128, KCH, 128], mm_dt, tag="wkv", name="wkv")
            nc.sync.dma_start(out=wkv_sb[:], in_=wkv.rearrange("(k p) m -> p k m", p=128))
            wo_sb = wp.tile([128, 4, HID], mm_dt, tag="wo", name="wo")
            for cc in range(4):
                w = 128 if cc < 3 else 64
                nc.sync.dma_start(out=wo_sb[0:w, cc, :], in_=wo[cc * 128:cc * 128 + w, :])
            cos_sb = wp.tile([128, ST, 32], F32, tag="cos", name="cos")
            nc.sync.dma_start(out=cos_sb[:], in_=cosr)
            sin_sb = wp.tile([128, ST, 32], F32, tag="sin", name="sin")
            nc.sync.dma_start(out=sin_sb[:], in_=sinr)
            triu_sb = wp.tile([128, 128], mm_dt, tag="triu", name="triu")
            nc.sync.dma_start(out=triu_sb[:], in_=triu)
            idn = wp.tile([128, 128], F32, tag="idn", name="idn")
            make_identity(nc, idn[:])
            idn_r = wp.tile([128, 128], mm_dt, tag="idnr", name="idnr")
            nc.vector.tensor_copy(idn_r[:], idn[:])
            idn_mm = idn_r[:]

            # qT / aoT in head-pair chunks: chunk j holds heads (2j, 2j+1)
            q_pair = [per.tile([128 if j < 3 else 64, s], mm_dt,
                               tag=f"qp{j}", name=f"qp{j}") for j in range(4)]
            aoT = [per.tile([128 if j < 3 else 64, s], mm_dt,
                            tag=f"ao{j}", name=f"ao{j}") for j in range(4)]
            kT2 = per.tile([128, s], mm_dt, tag="kT2", name="kT2")
            # merged kv staging: [k(64) | v(64) | ones(1)] per s-tile
            kv_all = per.tile([128, ST, 129], mm_dt, tag="kv_all", name="kv_all")
            if mm_dt == F16:
                nc.vector.memset(
                    kv_all[:, :, 128:129].bitcast(mybir.dt.uint16), 0x3C00)
            else:
                nc.vector.memset(kv_all[:, :, 128:129].bitcast(F32), 1.0)

            xT_sb = [xp.tile([128, s], mm_dt, tag=f"xT{k}", name=f"xT{k}")
                     for k in range(KCH)]
            for k in range(KCH):
                nc.sync.dma_start(out=xT_sb[k][:],
                                  in_=xT[k * 128:(k + 1) * 128, :])

            def _body():
                LOOKP = 2
                with tc.tile_pool(name="psA", bufs=1, space="PSUM") as psA:
                    # t-slot provider: one PSUM bank of 8 rotating [128,128]
                    # f16 quarter-slots, scoped per A-group (set by the loop).
                    tstate = {"pool": None, "slot": 0, "warm": 0}
                    t_bank = psA.tile([128, 8, 128], mm_dt, tag="t", name="t")

                    def _tslot():
                        sl = tstate["slot"]
                        tstate["slot"] = (sl + 1) % 8
                        return t_bank[:, sl, :]

                    def _emit_qtrans(st, q_rot):
                        for cc in range(4):
                            w = 128 if cc < 3 else 64
                            t_ps = _tslot()[0:w, :]
                            nc.tensor.transpose(t_ps, q_rot[:, cc * 128:cc * 128 + w],
                                                idn_mm)
                            nc.scalar.copy(
                                out=q_pair[cc][:, st * 128:(st + 1) * 128], in_=t_ps)

                    pend_a = []
                    pend_mul = []

                    def emit_proj(st):
                        q_ps = tstate["pool"].tile([128, GD], F32, tag="q",
                                                   name="q", bufs=2)
                        kv_ps = tstate["pool"].tile([128, 128], F32, tag="kv",
                                                    name="kv", bufs=1)
                        for kc in range(KCH):
                            lhsT = xT_sb[kc][:, st * 128:(st + 1) * 128]
                            nc.tensor.matmul(q_ps[:], lhsT, wq_sb[:, kc, :],
                                             start=(kc == 0), stop=(kc == KCH - 1))
                            nc.tensor.matmul(kv_ps[:], lhsT, wkv_sb[:, kc, :],
                                             start=(kc == 0), stop=(kc == KCH - 1))
                        # k, v staged first in one cast-copy so the single
                        # kv_ps buffer frees before the RoPE chain runs
                        nc.vector.tensor_copy(kv_all[:, st, 0:128], kv_ps[:])
                        # RoPE on q (natural layout, pairs = adjacent channels)
                        qv = q_ps[:].rearrange("p (h u two) -> p h u two", two=2, u=32)
                        e, o = qv[:, :, :, 0], qv[:, :, :, 1]
                        cb = _bc7(cos_sb[:, st, :])
                        sb_ = _bc7(sin_sb[:, st, :])
                        t1 = tmp.tile([128, HL, 32], F32, tag="t1", name="t1", bufs=1)
                        t2 = tmp.tile([128, HL, 32], F32, tag="t2", name="t2", bufs=1)
                        t3 = tmp.tile([128, HL, 32], F32, tag="t3", name="t3", bufs=1)
                        t4 = tmp.tile([128, HL, 32], F32, tag="t4", name="t4", bufs=1)
                        nc.vector.tensor_mul(t1[:], e, cb)
                        nc.vector.tensor_mul(t2[:], o, sb_)
                        nc.vector.tensor_mul(t3[:], e, sb_)
                        nc.vector.tensor_mul(t4[:], o, cb)
                        q_rot = tmp.tile([128, GD], mm_dt, tag="qrot", name="qrot",
                                           bufs=3)
                        qrv = q_rot[:].rearrange("p (h u two) -> p h u two", two=2, u=32)
                        nc.vector.tensor_sub(qrv[:, :, :, 0], t1[:], t2[:])
                        nc.vector.tensor_add(qrv[:, :, :, 1], t3[:], t4[:])
                        # transposes for the PREVIOUS s-tile go after this
                        # tile's projections so PE never waits on the RoPE DVE
                        pend_a.append((st, q_rot))
                        if len(pend_a) > 2:
                            _emit_qtrans(*pend_a.pop(0))

                    def emit_krope(blk):
                        kv4 = kv_all[:, blk * TPB:(blk + 1) * TPB, 0:64].rearrange(
                            "p s (u two) -> p s u two", two=2)
                        ke, ko = kv4[:, :, :, 0], kv4[:, :, :, 1]
                        k1 = tmp.tile([128, TPB, 32], F32, tag="k1", name="k1", bufs=1)
                        k2 = tmp.tile([128, TPB, 32], F32, tag="k2", name="k2", bufs=1)
                        k3 = tmp.tile([128, TPB, 32], F32, tag="k3", name="k3", bufs=1)
                        k4 = tmp.tile([128, TPB, 32], F32, tag="k4", name="k4", bufs=1)
                        cs = cos_sb[:, blk * TPB:(blk + 1) * TPB, :]
                        ss = sin_sb[:, blk * TPB:(blk + 1) * TPB, :]
                        nc.vector.tensor_mul(k1[:], ke, cs)
                        nc.vector.tensor_mul(k2[:], ko, ss)
                        nc.vector.tensor_mul(k3[:], ke, ss)
                        nc.vector.tensor_mul(k4[:], ko, cs)
                        nc.vector.tensor_sub(ke, k1[:], k2[:])
                        nc.vector.tensor_add(ko, k3[:], k4[:])

                    def emit_ktrans(blk):
                        for st in range(blk * TPB, (blk + 1) * TPB):
                            t_ps = _tslot()[0:64, :]
                            nc.tensor.transpose(t_ps, kv_all[:, st, 0:64], idn_mm)
                            nc.vector.tensor_copy(kT2[0:64, st * 128:(st + 1) * 128], t_ps)
                            nc.vector.tensor_copy(kT2[64:128, st * 128:(st + 1) * 128], t_ps)

                    def emit_attn(qc):
                        # k-tiles in pairs sharing a [128,1024] psum + one exp;
                        # LOOKP pairs in flight so PE stays ahead of ACT.
                        # Unwritten psum regions of partial (diagonal) tiles
                        # hold stale garbage whose exp is never consumed.
                        with tc.tile_pool(name="psB", bufs=1, space="PSUM") as psB, \
                             tc.tile_pool(name="psO", bufs=1, space="PSUM") as psO:
                            for h in range(HL):
                                half = (h % 2) * 64
                                qsrc = q_pair[h // 2]
                                o_ps = psO.tile([65, 512], F32, tag="o", name="o")
                                nkt = 4 * (qc + 1)
                                npair = (nkt + 1) // 2

                                def emit_pair(pi, qc=qc, half=half, qsrc=qsrc, nkt=nkt):
                                    s_ps = psB.tile([128, 1024], F32, tag="sp",
                                                    name="sp", bufs=3)
                                    ex = expp.tile([128, 1024], mm_dt, tag="ex",
                                                   name="ex", bufs=4)
                                    info = []
                                    for j in (0, 1):
                                        kt = 2 * pi + j
                                        if kt >= nkt:
                                            break
                                        rrel = kt - 4 * qc
                                        off = 128 * rrel if rrel >= 0 else 0
                                        N = 512 - off
                                        nc.tensor.matmul(
                                            s_ps[:, 512 * j + off:512 * (j + 1)],
                                            kT2[half:half + 64, kt * 128:(kt + 1) * 128],
                                            qsrc[half:half + 64,
                                                 qc * 512 + off:(qc + 1) * 512],
                                            start=True, stop=True)
                                        info.append((kt, 512 * j + off, off, N, rrel))
                                    # exp over each contiguous written run (a
                                    # diagonal second tile leaves an unwritten gap)
                                    runs = []
                                    for kt, base, off, N, rrel in info:
                                        if runs and runs[-1][1] == base:
                                            runs[-1][1] = base + N
                                        else:
                                            runs.append([base, base + N])
                                    for lo, hi in runs:
                                        nc.scalar.activation(out=ex[:, lo:hi],
                                                             in_=s_ps[:, lo:hi],
                                                             func=AF.Exp)
                                    for kt, base, off, N, rrel in info:
                                        if rrel >= 0:
                                            nc.vector.tensor_mul(
                                                ex[:, base:base + 128],
                                                ex[:, base:base + 128], triu_sb[:])
                                    return ex, info

                                pend = {}
                                for pi in range(min(LOOKP + 1, npair)):
                                    pend[pi] = emit_pair(pi)
                                for pi in range(npair):
                                    nxt = pi + LOOKP + 1
                                    if nxt < npair:
                                        pend[nxt] = emit_pair(nxt)
                                    ex, info = pend.pop(pi)
                                    for kt, base, off, N, rrel in info:
                                        nc.tensor.matmul(
                                            o_ps[:, off:512], kv_all[:, kt, 64:129],
                                            ex[:, base:base + N],
                                            start=(kt == 0), stop=(kt == nkt - 1))
                                    wsl = 6 + (tstate["warm"] % 2)
                                    tstate["warm"] += 1
                                    nc.tensor.matmul(
                                        t_bank[0:64, wsl, 0:64].bitcast(F32),
                                        idn[0:64, 0:64], idn[0:64, 0:32],
                                        start=True, stop=True)

                                # ---- per-chunk softmax normalization ----
                                # stage raw attn + den row to SBUF (frees the
                                # psum fast), fast-reciprocal of the den row,
                                # DRAM roundtrip for the partition-broadcast
                                # (legal on DRAM-source DMAs, pipelines on the
                                # DMA engines), one multiply into aoT.
                                araw = rb.tile([64, 512], mm_dt, tag="araw",
                                               name="araw", bufs=8)
                                nc.vector.tensor_copy(araw[:], o_ps[0:64, :])
                                den_row = rb.tile([1, 512], F32, tag="den",
                                                  name="den", bufs=2)
                                nc.vector.tensor_copy(den_row[:], o_ps[64:65, :])
                                rec_sb = rb.tile([1, 512], F32, tag="rec",
                                                 name="rec", bufs=2)
                                nc.vector.reciprocal_approx_fast(rec_sb[:], den_row[:])
                                nc.sync.dma_start(
                                    out=dram_rec[h:h + 1, qc * 512:(qc + 1) * 512],
                                    in_=rec_sb[:])
                                rbF = rb.tile([64, 512], F32, tag="rb", name="rb",
                                              bufs=8)
                                nc.sync.dma_start(out=rbF[:], in_=bass.AP(
                                    tensor=dram_rec.tensor, offset=h * s + qc * 512,
                                    ap=[[0, 64], [1, 512]]))
                                pend_mul.append((
                                    aoT[h // 2][half:half + 64,
                                                qc * 512:(qc + 1) * 512],
                                    araw, rbF))

                    def _flush_muls():
                        for dst, a_t, r_t in pend_mul:
                            nc.vector.tensor_mul(dst, a_t[:], r_t[:])
                        pend_mul.clear()

                    def emit_out(blk):
                        with tc.tile_pool(name="psC", bufs=2, space="PSUM") as psC:
                            for st in range(blk * TPB, (blk + 1) * TPB):
                                y_ps = psC.tile([128, 1024], F32, tag="y", name="y")
                                for cc in range(4):
                                    w = 128 if cc < 3 else 64
                                    lhsT = aoT[cc][0:w, st * 128:(st + 1) * 128]
                                    nc.tensor.matmul(y_ps[:, 0:512], lhsT,
                                                     wo_sb[0:w, cc, 0:512],
                                                     start=(cc == 0), stop=(cc == 3))
                                    nc.tensor.matmul(y_ps[:, 512:896], lhsT,
                                                     wo_sb[0:w, cc, 512:896],
                                                     start=(cc == 0), stop=(cc == 3))
                                y_sb = tmp.tile([128, HID], F32, tag="ysb", name="ysb")
                                nc.scalar.copy(out=y_sb[:], in_=y_ps[:, 0:896])
                                nc.sync.dma_start(out=y[st * 128:(st + 1) * 128, :],
                                                  in_=y_sb[:])

                    # rolling schedule: projections run two tiles ahead of the
                    # block boundary so the PE has independent matmuls while
                    # the DVE drains the block's RoPE chain; C(blk-1) lands
                    # after B(blk) so its aoT inputs are long settled. The
                    # transpose PSUM bank (psT) is scoped per A-group so it
                    # time-shares banks with the B/C pools.
                    for blk in range(QC):
                        with tc.tile_pool(name="psT", bufs=1, space="PSUM") as psT:
                            tstate["pool"] = psT
                            _flush_muls()
                            if blk == 0:
                                for st in range(TPB):
                                    emit_proj(st)
                            else:
                                emit_proj(blk * TPB + 2)
                                emit_proj(blk * TPB + 3)
                            emit_krope(blk)
                            nxt = (blk + 1) * TPB
                            if nxt < ST:
                                emit_proj(nxt)
                                emit_proj(nxt + 1)
                            while pend_a and pend_a[0][0] < nxt:
                                _emit_qtrans(*pend_a.pop(0))
                            emit_ktrans(blk)
                        emit_attn(blk)
                        if blk > 0:
                            emit_out(blk - 1)
                    _flush_muls()
                    emit_out(QC - 1)

            if reps > 1:
                with tc.For_i(0, reps, 1):
                    _body()
            else:
                _body()

    nc.compile()
    return nc


# ---------------------------------------------------------------------------
# host-side sharding + execution
# ---------------------------------------------------------------------------

def round_f32r(a):
    """Round fp32 array to fp32r (RNE to 11 mantissa bits) -- bit-exact match
    of the hardware's casting DMA, verified by SBUF readback."""
    b = np.ascontiguousarray(a, dtype=np.float32).view(np.uint32)
    lsb = (b >> np.uint32(12)) & np.uint32(1)
    r = ((b + np.uint32(0x7FF) + lsb) & np.uint32(0xFFFFF000))
    return r.view(np.float32)


MM_DT = {"f32r": F32R, "f16": F16, "f32": F32}[os.environ.get("MM_DT", "f16")]


def _cvt(a, mm_dt):
    if mm_dt == F16:
        return np.ascontiguousarray(np.asarray(a, dtype=np.float32)).astype(np.float16)
    if mm_dt == F32R:
        return round_f32r(a)
    return np.ascontiguousarray(a, dtype=np.float32)


def make_in_maps(x, freqs_cos, freqs_sin, Wq, Wk, Wv, Wo, s=S, mm_dt=None):
    if mm_dt is None:
        mm_dt = MM_DT
    ST = s // 128
    scale = 1.0 / math.sqrt(D)
    cosr = np.ascontiguousarray(
        np.asarray(freqs_cos).reshape(ST, 128, 32).transpose(1, 0, 2)).astype(np.float32)
    sinr = np.ascontiguousarray(
        np.asarray(freqs_sin).reshape(ST, 128, 32).transpose(1, 0, 2)).astype(np.float32)
    triu = _cvt(np.triu(np.ones((128, 128), dtype=np.float32)), mm_dt)
    in_maps = []
    for c in range(N_CORES):
        b, g = c // 2, c % 2
        in_maps.append({
            "xT": _cvt(np.asarray(x)[b].T, mm_dt),
            "wq": _cvt(np.asarray(Wq)[:, g * GD:(g + 1) * GD] * scale, mm_dt),
            "wkv": _cvt(np.concatenate(
                [np.asarray(Wk)[:, g * D:(g + 1) * D],
                 np.asarray(Wv)[:, g * D:(g + 1) * D]], axis=1), mm_dt),
            "wo": _cvt(np.asarray(Wo)[g * GD:(g + 1) * GD, :], mm_dt),
            "cosr": cosr, "sinr": sinr, "triu": triu,
        })
    return in_maps


_RUNNER = None


class _Runner:
    """Minimal SPMD executor over axon PJRT (self-contained copy)."""

    def __init__(self, nc, n_cores):
        import jax
        from jax.sharding import Mesh, PartitionSpec, NamedSharding
        from jax.experimental.shard_map import shard_map
        from concourse.bass2jax import (_bass_exec_p, install_neuronx_cc_hook,
                                        partition_id_tensor)
        install_neuronx_cc_hook()
        self.jax = jax
        self.n_cores = n_cores
        partition_name = (nc.partition_id_tensor.name
                          if nc.partition_id_tensor else None)
        in_names, out_names, out_avals = [], [], []
        for alloc in nc.m.functions[0].allocations:
            if not isinstance(alloc, mybir.MemoryLocationSet):
                continue
            name = alloc.memorylocations[0].name
            if alloc.kind == "ExternalInput":
                if name != partition_name:
                    in_names.append(name)
            elif alloc.kind == "ExternalOutput":
                out_names.append(name)
                out_avals.append(jax.core.ShapedArray(
                    tuple(alloc.tensor_shape), mybir.dt.np(alloc.dtype)))
        self.in_names, self.out_names, self.out_avals = in_names, out_names, out_avals
        n_params, n_outs = len(in_names), len(out_avals)
        all_names = in_names + out_names
        if partition_name is not None:
            all_names.append(partition_name)

        def _body(*args):
            operands = list(args)
            if partition_name is not None:
                operands.append(partition_id_tensor())
            return tuple(_bass_exec_p.bind(
                *operands, out_avals=tuple(out_avals), in_names=tuple(all_names),
                out_names=tuple(out_names), lowering_input_output_aliases=(),
                sim_require_finite=False, sim_require_nnan=False, nc=nc))

        devices = jax.devices()[:n_cores]
        self.mesh = Mesh(np.asarray(devices), ("core",))
        self.sharding = NamedSharding(self.mesh, PartitionSpec("core"))
        in_specs = (PartitionSpec("core"),) * (n_params + n_outs)
        out_specs = (PartitionSpec("core"),) * n_outs
        self.fn = jax.jit(
            shard_map(_body, mesh=self.mesh, in_specs=in_specs,
                      out_specs=out_specs, check_rep=False),
            donate_argnums=tuple(range(n_params, n_params + n_outs)),
            keep_unused=True)
        zshapes = [(n_cores * a.shape[0], *a.shape[1:]) for a in out_avals]
        zdtypes = [a.dtype for a in out_avals]
        self.make_zeros = jax.jit(
            lambda: tuple(jax.numpy.zeros(sh, dt)
                          for sh, dt in zip(zshapes, zdtypes)),
            out_shardings=tuple(self.sharding for _ in zshapes))

    def prep(self, in_maps):
        return [self.jax.device_put(
            np.concatenate([np.asarray(in_maps[c][n]) for c in range(self.n_cores)],
                           axis=0), self.sharding)
            for n in self.in_names]

    def run(self, dev_in):
        return self.fn(*dev_in, *self.make_zeros())

    def split(self, outs):
        res = []
        for c in range(self.n_cores):
            res.append({n: np.asarray(outs[i]).reshape(
                self.n_cores, *self.out_avals[i].shape)[c]
                for i, n in enumerate(self.out_names)})
        return res


def get_runner():
    global _RUNNER
    if _RUNNER is None:
        _RUNNER = _Runner(build(), N_CORES)
    return _RUNNER


def kernel(x, freqs_cos, freqs_sin, mask, Wq, Wk, Wv, Wo):
    x = np.asarray(x, dtype=np.float32)
    in_maps = make_in_maps(np.asarray(x), np.asarray(freqs_cos),
                           np.asarray(freqs_sin), np.asarray(Wq),
                           np.asarray(Wk), np.asarray(Wv), np.asarray(Wo))
    r = get_runner()
    outs = r.run(r.prep(in_maps))
    res = r.split(outs)
    out = np.empty((B, S, HID), dtype=np.float32)
    for b in range(B):
        out[b] = res[2 * b]["y"] + res[2 * b + 1]["y"]
    return out


# revision 29
# speedup vs baseline: 1.5661x; 1.5661x over previous
"""Trainium2 Bass kernel for GQA attention (B=4, S=2048, HID=896, H=14, KV=2, D=64).

Sharding: 8 cores = 4 batches x 2 KV-head groups. Core c handles batch c//2,
query heads [g*7, (g+1)*7) with g = c%2 (exactly one KV head per core thanks to
GQA group structure). Each core computes its 448-channel slice of attn output
and the partial output projection y_g = ao_g @ Wo[g*448:(g+1)*448, :]; the host
sums the two partials per batch.

Per-core pipeline, interleaved in 4 super-blocks of 4 s-tiles each so the PE
never idles long enough for the HAM clock gate to re-throttle to 4/8:

  block i:
    A)  s-tile projections q = x@Wq_g (448 ch), kv = x@[Wk_g|Wv_g] (128 ch),
        emitted on a rolling schedule TWO tiles ahead of the block boundary so
        the PE has independent matmul work while the DVE RoPE/staging chain
        for the block drains. RoPE on q in natural [s, ch] layout on DVE; k
        RoPE'd in one batched DVE pass per block; PE-transposes to
        channel-major (q_pair head-pair tensors, kT2 duplicated into both
        partition halves).
    B)  q-chunk qc=i for ALL 7 heads: scoresT[k,q] = kT.T @ qT, structural
        causality (k-tiles <= diagonal; diagonal tiles exp'd then masked with
        a triangular 0/1 multiply). k-tile pairs share one [128,1024] PSUM +
        one exp; LOOKP pairs in flight. attn@[v|1] accumulates into PSUM
        [65,512]; row 64 = softmax denominators. Evacuation is per chunk: raw
        attn rows staged to SBUF, DVE fast-reciprocal of the den row, DRAM
        partition-broadcast roundtrip, one DVE multiply into aoT.
    C)  y s-tiles of block i-1 (emitted after B of block i, so the per-chunk
        normalization DMA chains have long resolved): y = ao @ Wo_g,
        PSUM -> SBUF -> DRAM. C(3) runs at the very end, shielded by C(2).

The causal mask input is never loaded: exp(-1e9 + s) == 0.0 exactly in fp32,
so structural masking matches the reference's additive mask bit-for-bit.

mm_dt selects the matmul dtype: float32r (2 PE cycles/row, ~1.4e-4 rounding =
RNE to 11 mantissa bits, replicated host-side) or float16 (1 cycle/row, ~2x
the rounding error) or float32 (exact, 4 cycles/row).
"""
import math
import os
import numpy as np

import concourse.bass as bass
import concourse.mybir as mybir
import concourse.tile as tile
from concourse import bacc
from concourse.masks import make_identity

F32 = mybir.dt.float32
F32R = mybir.dt.float32r
F16 = mybir.dt.float16
AF = mybir.ActivationFunctionType

B, S, HID = 4, 2048, 896
H, KV, D = 14, 2, 64
HL = H // KV          # 7 local query heads per core
GD = HL * D           # 448 local channels
KCH = HID // 128      # 7 contraction chunks
N_CORES = 8


def _bc7(ap_small):
    """[128, 32] cos/sin slice -> broadcast over the 7 heads: [128, 7, 32]."""
    return bass.AP(
        tensor=ap_small.tensor,
        offset=ap_small.offset,
        ap=[list(ap_small.ap[0]), [0, HL], list(ap_small.ap[1])],
    )


def build(s=S, mm_dt=None, reps=1, phases="A2BC"):
    if mm_dt is None:
        mm_dt = MM_DT
    ST = s // 128           # s-tiles
    QC = s // 512           # q chunks (also the number of super-blocks)
    TPB = ST // QC          # s-tiles per super-block (4)
    nc = bacc.Bacc("TRN2", target_bir_lowering=False, debug=False,
                   num_devices=N_CORES)

    xT = nc.dram_tensor("xT", [HID, s], mm_dt, kind="ExternalInput").ap()
    wq = nc.dram_tensor("wq", [HID, GD], mm_dt, kind="ExternalInput").ap()
    wkv = nc.dram_tensor("wkv", [HID, 128], mm_dt, kind="ExternalInput").ap()
    wo = nc.dram_tensor("wo", [GD, HID], mm_dt, kind="ExternalInput").ap()
    cosr = nc.dram_tensor("cosr", [128, ST, 32], F32, kind="ExternalInput").ap()
    sinr = nc.dram_tensor("sinr", [128, ST, 32], F32, kind="ExternalInput").ap()
    triu = nc.dram_tensor("triu", [128, 128], mm_dt, kind="ExternalInput").ap()
    y = nc.dram_tensor("y", [s, HID], F32, kind="ExternalOutput").ap()
    dram_rec = nc.dram_tensor("dram_rec", [HL, s], F32).ap()

    with tile.TileContext(nc) as tc:
        with (
            tc.tile_pool(name="wp", bufs=1) as wp,
            tc.tile_pool(name="per", bufs=1) as per,
            tc.tile_pool(name="tmp", bufs=2) as tmp,
            tc.tile_pool(name="expp", bufs=1) as expp,
            tc.tile_pool(name="rb", bufs=1) as rb,
            tc.tile_pool(name="xp", bufs=1) as xp,
        ):
            # ---- rep-invariant loads + persistent intermediates, hoisted out
            # of the rep loop: weights/tables/x stay resident in SBUF, so the
            # steady-state rep starts computing immediately ----
            wq_sb = wp.tile([128, KCH, GD], mm_dt, tag="wq", name="wq")
            nc.sync.dma_start(out=wq_sb[:], in_=wq.rearrange("(k p) m -> p k m", p=128))
            wkv_sb = wp.tile([128, KCH, 128], mm_dt, tag="wkv", name="wkv")
            nc.sync.dma_start(out=wkv_sb[:], in_=wkv.rearrange("(k p) m -> p k m", p=128))
            wo_sb = wp.tile([128, 4, HID], mm_dt, tag="wo", name="wo")
            for cc in range(4):
                w = 128 if cc < 3 else 64
                nc.sync.dma_start(out=wo_sb[0:w, cc, :], in_=wo[cc * 128:cc * 128 + w, :])
            cos_sb = wp.tile([128, ST, 32], F32, tag="cos", name="cos")
            nc.sync.dma_start(out=cos_sb[:], in_=cosr)
            sin_sb = wp.tile([128, ST, 32], F32, tag="sin", name="sin")
            nc.sync.dma_start(out=sin_sb[:], in_=sinr)
            triu_sb = wp.tile([128, 128], mm_dt, tag="triu", name="triu")
            nc.sync.dma_start(out=triu_sb[:], in_=triu)
            idn = wp.tile([128, 128], F32, tag="idn", name="idn")
            make_identity(nc, idn[:])
            idn_r = wp.tile([128, 128], mm_dt, tag="idnr", name="idnr")
            nc.vector.tensor_copy(idn_r[:], idn[:])
            idn_mm = idn_r[:]

            # qT / aoT in head-pair chunks: chunk j holds heads (2j, 2j+1)
            q_pair = [per.tile([128 if j < 3 else 64, s], mm_dt,
                               tag=f"qp{j}", name=f"qp{j}") for j in range(4)]
            aoT = [per.tile([128 if j < 3 else 64, s], mm_dt,
                            tag=f"ao{j}", name=f"ao{j}") for j in range(4)]
            kT2 = per.tile([128, s], mm_dt, tag="kT2", name="kT2")
            # merged kv staging: [k(64) | v(64) | ones(1)] per s-tile
            kv_all = per.tile([128, ST, 129], mm_dt, tag="kv_all", name="kv_all")
            if mm_dt == F16:
                nc.vector.memset(
                    kv_all[:, :, 128:129].bitcast(mybir.dt.uint16), 0x3C00)
            else:
                nc.vector.memset(kv_all[:, :, 128:129].bitcast(F32), 1.0)

            xT_sb = [xp.tile([128, s], mm_dt, tag=f"xT{k}", name=f"xT{k}")
                     for k in range(KCH)]
            for k in range(KCH):
                nc.sync.dma_start(out=xT_sb[k][:],
                                  in_=xT[k * 128:(k + 1) * 128, :])

            def _body():
                LOOKP = 2
                with tc.tile_pool(name="psA", bufs=1, space="PSUM") as psA:
                    # t-slot provider: one PSUM bank of 8 rotating [128,128]
                    # f16 quarter-slots, scoped per A-group (set by the loop).
                    tstate = {"pool": None, "slot": 0, "warm": 0}
                    t_bank = psA.tile([128, 8, 128], mm_dt, tag="t", name="t")

                    def _tslot():
                        sl = tstate["slot"]
                        tstate["slot"] = (sl + 1) % 8
                        return t_bank[:, sl, :]

                    def _emit_qtrans(st, q_rot):
                        for cc in range(4):
                            w = 128 if cc < 3 else 64
                            t_ps = _tslot()[0:w, :]
                            nc.tensor.transpose(t_ps, q_rot[:, cc * 128:cc * 128 + w],
                                                idn_mm)
                            nc.scalar.copy(
                                out=q_pair[cc][:, st * 128:(st + 1) * 128], in_=t_ps)

                    pend_a = []
                    pend_mul = []

                    def emit_proj(st):
                        q_ps = tstate["pool"].tile([128, GD], F32, tag="q",
                                                   name="q", bufs=2)
                        kv_ps = tstate["pool"].tile([128, 128], F32, tag="kv",
                                                    name="kv", bufs=1)
                        for kc in range(KCH):
                            lhsT = xT_sb[kc][:, st * 128:(st + 1) * 128]
                            nc.tensor.matmul(q_ps[:], lhsT, wq_sb[:, kc, :],
                                             start=(kc == 0), stop=(kc == KCH - 1))
                            nc.tensor.matmul(kv_ps[:], lhsT, wkv_sb[:, kc, :],
                                             start=(kc == 0), stop=(kc == KCH - 1))
                        # k, v staged first in one cast-copy so the single
                        # kv_ps buffer frees before the RoPE chain runs
                        nc.vector.tensor_copy(kv_all[:, st, 0:128], kv_ps[:])
                        # RoPE on q (natural layout, pairs = adjacent channels)
                        qv = q_ps[:].rearrange("p (h u two) -> p h u two", two=2, u=32)
                        e, o = qv[:, :, :, 0], qv[:, :, :, 1]
                        cb = _bc7(cos_sb[:, st, :])
                        sb_ = _bc7(sin_sb[:, st, :])
                        t1 = tmp.tile([128, HL, 32], F32, tag="t1", name="t1", bufs=1)
                        t2 = tmp.tile([128, HL, 32], F32, tag="t2", name="t2", bufs=1)
                        t3 = tmp.tile([128, HL, 32], F32, tag="t3", name="t3", bufs=1)
                        t4 = tmp.tile([128, HL, 32], F32, tag="t4", name="t4", bufs=1)
                        nc.vector.tensor_mul(t1[:], e, cb)
                        nc.vector.tensor_mul(t2[:], o, sb_)
                        nc.vector.tensor_mul(t3[:], e, sb_)
                        nc.vector.tensor_mul(t4[:], o, cb)
                        q_rot = tmp.tile([128, GD], mm_dt, tag="qrot", name="qrot",
                                           bufs=3)
                        qrv = q_rot[:].rearrange("p (h u two) -> p h u two", two=2, u=32)
                        nc.vector.tensor_sub(qrv[:, :, :, 0], t1[:], t2[:])
                        nc.vector.tensor_add(qrv[:, :, :, 1], t3[:], t4[:])
                        # transposes for the PREVIOUS s-tile go after this
                        # tile's projections so PE never waits on the RoPE DVE
                        pend_a.append((st, q_rot))
                        if len(pend_a) > 2:
                            _emit_qtrans(*pend_a.pop(0))

                    def emit_krope(blk):
                        kv4 = kv_all[:, blk * TPB:(blk + 1) * TPB, 0:64].rearrange(
                            "p s (u two) -> p s u two", two=2)
                        ke, ko = kv4[:, :, :, 0], kv4[:, :, :, 1]
                        k1 = tmp.tile([128, TPB, 32], F32, tag="k1", name="k1", bufs=1)
                        k2 = tmp.tile([128, TPB, 32], F32, tag="k2", name="k2", bufs=1)
                        k3 = tmp.tile([128, TPB, 32], F32, tag="k3", name="k3", bufs=1)
                        k4 = tmp.tile([128, TPB, 32], F32, tag="k4", name="k4", bufs=1)
                        cs = cos_sb[:, blk * TPB:(blk + 1) * TPB, :]
                        ss = sin_sb[:, blk * TPB:(blk + 1) * TPB, :]
                        nc.vector.tensor_mul(k1[:], ke, cs)
                        nc.vector.tensor_mul(k2[:], ko, ss)
                        nc.vector.tensor_mul(k3[:], ke, ss)
                        nc.vector.tensor_mul(k4[:], ko, cs)
                        nc.vector.tensor_sub(ke, k1[:], k2[:])
                        nc.vector.tensor_add(ko, k3[:], k4[:])

                    def emit_ktrans(blk):
                        for st in range(blk * TPB, (blk + 1) * TPB):
                            t_ps = _tslot()[0:64, :]
                            nc.tensor.transpose(t_ps, kv_all[:, st, 0:64], idn_mm)
                            nc.vector.tensor_copy(kT2[0:64, st * 128:(st + 1) * 128], t_ps)
                            nc.vector.tensor_copy(kT2[64:128, st * 128:(st + 1) * 128], t_ps)

                    def emit_attn(qc):
                        # k-tiles in pairs sharing a [128,1024] psum + one exp;
                        # LOOKP pairs in flight so PE stays ahead of ACT.
                        # Unwritten psum regions of partial (diagonal) tiles
                        # hold stale garbage whose exp is never consumed.
                        with tc.tile_pool(name="psB", bufs=1, space="PSUM") as psB, \
                             tc.tile_pool(name="psO", bufs=1, space="PSUM") as psO:
                            for h in range(HL):
                                half = (h % 2) * 64
                                qsrc = q_pair[h // 2]
                                o_ps = psO.tile([65, 512], F32, tag="o", name="o")
                                nkt = 4 * (qc + 1)
                                npair = (nkt + 1) // 2

                                def emit_pair(pi, qc=qc, half=half, qsrc=qsrc, nkt=nkt):
                                    s_ps = psB.tile([128, 1024], F32, tag="sp",
                                                    name="sp", bufs=3)
                                    ex = expp.tile([128, 1024], mm_dt, tag="ex",
                                                   name="ex", bufs=4)
                                    info = []
                                    for j in (0, 1):
                                        kt = 2 * pi + j
                                        if kt >= nkt:
                                            break
                                        rrel = kt - 4 * qc
                                        off = 128 * rrel if rrel >= 0 else 0
                                        N = 512 - off
                                        nc.tensor.matmul(
                                            s_ps[:, 512 * j + off:512 * (j + 1)],
                                            kT2[half:half + 64, kt * 128:(kt + 1) * 128],
                                            qsrc[half:half + 64,
                                                 qc * 512 + off:(qc + 1) * 512],
                                            start=True, stop=True)
                                        info.append((kt, 512 * j + off, off, N, rrel))
                                    # exp over each contiguous written run (a
                                    # diagonal second tile leaves an unwritten gap)
                                    runs = []
                                    for kt, base, off, N, rrel in info:
                                        if runs and runs[-1][1] == base:
                                            runs[-1][1] = base + N
                                        else:
                                            runs.append([base, base + N])
                                    for lo, hi in runs:
                                        nc.scalar.activation(out=ex[:, lo:hi],
                                                             in_=s_ps[:, lo:hi],
                                                             func=AF.Exp)
                                    for kt, base, off, N, rrel in info:
                                        if rrel >= 0:
                                            nc.vector.tensor_mul(
                                                ex[:, base:base + 128],
                                                ex[:, base:base + 128], triu_sb[:])
                                    return ex, info

                                pend = {}
                                for pi in range(min(LOOKP + 1, npair)):
                                    pend[pi] = emit_pair(pi)
                                for pi in range(npair):
                                    nxt = pi + LOOKP + 1
                                    if nxt < npair:
                                        pend[nxt] = emit_pair(nxt)
                                    ex, info = pend.pop(pi)
                                    for kt, base, off, N, rrel in info:
                                        nc.tensor.matmul(
                                            o_ps[:, off:512], kv_all[:, kt, 64:129],
                                            ex[:, base:base + N],
                                            start=(kt == 0), stop=(kt == nkt - 1))

                                # ---- per-chunk softmax normalization ----
                                # stage raw attn + den row to SBUF (frees the
                                # psum fast), fast-reciprocal of the den row,
                                # DRAM roundtrip for the partition-broadcast
                                # (legal on DRAM-source DMAs, pipelines on the
                                # DMA engines), one multiply into aoT.
                                araw = rb.tile([64, 512], mm_dt, tag="araw",
                                               name="araw", bufs=8)
                                nc.vector.tensor_copy(araw[:], o_ps[0:64, :])
                                den_row = rb.tile([1, 512], F32, tag="den",
                                                  name="den", bufs=2)
                                nc.vector.tensor_copy(den_row[:], o_ps[64:65, :])
                                rec_sb = rb.tile([1, 512], F32, tag="rec",
                                                 name="rec", bufs=2)
                                nc.vector.reciprocal_approx_fast(rec_sb[:], den_row[:])
                                nc.sync.dma_start(
                                    out=dram_rec[h:h + 1, qc * 512:(qc + 1) * 512],
                                    in_=rec_sb[:])
                                rbF = rb.tile([64, 512], F32, tag="rb", name="rb",
                                              bufs=8)
                                nc.sync.dma_start(out=rbF[:], in_=bass.AP(
                                    tensor=dram_rec.tensor, offset=h * s + qc * 512,
                                    ap=[[0, 64], [1, 512]]))
                                pend_mul.append((
                                    aoT[h // 2][half:half + 64,
                                                qc * 512:(qc + 1) * 512],
                                    araw, rbF))

                    def _flush_muls():
                        for dst, a_t, r_t in pend_mul:
                            nc.vector.tensor_mul(dst, a_t[:], r_t[:])
                        pend_mul.clear()

                    def emit_out(blk):
                        with tc.tile_pool(name="psC", bufs=2, space="PSUM") as psC:
                            for st in range(blk * TPB, (blk + 1) * TPB):
                                y_ps = psC.tile([128, 1024], F32, tag="y", name="y")
                                for cc in range(4):
                                    w = 128 if cc < 3 else 64
                                    lhsT = aoT[cc][0:w, st * 128:(st + 1) * 128]
                                    nc.tensor.matmul(y_ps[:, 0:512], lhsT,
                                                     wo_sb[0:w, cc, 0:512],
                                                     start=(cc == 0), stop=(cc == 3))
                                    nc.tensor.matmul(y_ps[:, 512:896], lhsT,
                                                     wo_sb[0:w, cc, 512:896],
                                                     start=(cc == 0), stop=(cc == 3))
                                y_sb = tmp.tile([128, HID], F32, tag="ysb", name="ysb")
                                nc.scalar.copy(out=y_sb[:], in_=y_ps[:, 0:896])
                                nc.sync.dma_start(out=y[st * 128:(st + 1) * 128, :],
                                                  in_=y_sb[:])

                    # rolling schedule: projections run two tiles ahead of the
                    # block boundary so the PE has independent matmuls while
                    # the DVE drains the block's RoPE chain; C(blk-1) lands
                    # after B(blk) so its aoT inputs are long settled. The
                    # transpose PSUM bank (psT) is scoped per A-group so it
                    # time-shares banks with the B/C pools.
                    for blk in range(QC):
                        with tc.tile_pool(name="psT", bufs=1, space="PSUM") as psT:
                            tstate["pool"] = psT
                            _flush_muls()
                            if blk == 0:
                                for st in range(TPB):
                                    emit_proj(st)
                            else:
                                emit_proj(blk * TPB + 2)
                                emit_proj(blk * TPB + 3)
                            emit_krope(blk)
                            nxt = (blk + 1) * TPB
                            if nxt < ST:
                                emit_proj(nxt)
                                emit_proj(nxt + 1)
                            while pend_a and pend_a[0][0] < nxt:
                                _emit_qtrans(*pend_a.pop(0))
                            emit_ktrans(blk)
                        emit_attn(blk)
                        if blk > 0:
                            emit_out(blk - 1)
                    _flush_muls()
                    emit_out(QC - 1)

            if reps > 1:
                with tc.For_i(0, reps, 1):
                    _body()
            else:
                _body()

    nc.compile()
    return nc


# ---------------------------------------------------------------------------
# host-side sharding + execution
# ---------------------------------------------------------------------------

def round_f32r(a):
    """Round fp32 array to fp32r (RNE to 11 mantissa bits) -- bit-exact match
    of the hardware's casting DMA, verified by SBUF readback."""
    b = np.ascontiguousarray(a, dtype=np.float32).view(np.uint32)
    lsb = (b >> np.uint32(12)) & np.uint32(1)
    r = ((b + np.uint32(0x7FF) + lsb) & np.uint32(0xFFFFF000))
    return r.view(np.float32)


MM_DT = {"f32r": F32R, "f16": F16, "f32": F32}[os.environ.get("MM_DT", "f16")]


def _cvt(a, mm_dt):
    if mm_dt == F16:
        return np.ascontiguousarray(np.asarray(a, dtype=np.float32)).astype(np.float16)
    if mm_dt == F32R:
        return round_f32r(a)
    return np.ascontiguousarray(a, dtype=np.float32)


def make_in_maps(x, freqs_cos, freqs_sin, Wq, Wk, Wv, Wo, s=S, mm_dt=None):
    if mm_dt is None:
        mm_dt = MM_DT
    ST = s // 128
    scale = 1.0 / math.sqrt(D)
    cosr = np.ascontiguousarray(
        np.asarray(freqs_cos).reshape(ST, 128, 32).transpose(1, 0, 2)).astype(np.float32)
    sinr = np.ascontiguousarray(
        np.asarray(freqs_sin).reshape(ST, 128, 32).transpose(1, 0, 2)).astype(np.float32)
    triu = _cvt(np.triu(np.ones((128, 128), dtype=np.float32)), mm_dt)
    in_maps = []
    for c in range(N_CORES):
        b, g = c // 2, c % 2
        in_maps.append({
            "xT": _cvt(np.asarray(x)[b].T, mm_dt),
            "wq": _cvt(np.asarray(Wq)[:, g * GD:(g + 1) * GD] * scale, mm_dt),
            "wkv": _cvt(np.concatenate(
                [np.asarray(Wk)[:, g * D:(g + 1) * D],
                 np.asarray(Wv)[:, g * D:(g + 1) * D]], axis=1), mm_dt),
            "wo": _cvt(np.asarray(Wo)[g * GD:(g + 1) * GD, :], mm_dt),
            "cosr": cosr, "sinr": sinr, "triu": triu,
        })
    return in_maps


_RUNNER = None


class _Runner:
    """Minimal SPMD executor over axon PJRT (self-contained copy)."""

    def __init__(self, nc, n_cores):
        import jax
        from jax.sharding import Mesh, PartitionSpec, NamedSharding
        from jax.experimental.shard_map import shard_map
        from concourse.bass2jax import (_bass_exec_p, install_neuronx_cc_hook,
                                        partition_id_tensor)
        install_neuronx_cc_hook()
        self.jax = jax
        self.n_cores = n_cores
        partition_name = (nc.partition_id_tensor.name
                          if nc.partition_id_tensor else None)
        in_names, out_names, out_avals = [], [], []
        for alloc in nc.m.functions[0].allocations:
            if not isinstance(alloc, mybir.MemoryLocationSet):
                continue
            name = alloc.memorylocations[0].name
            if alloc.kind == "ExternalInput":
                if name != partition_name:
                    in_names.append(name)
            elif alloc.kind == "ExternalOutput":
                out_names.append(name)
                out_avals.append(jax.core.ShapedArray(
                    tuple(alloc.tensor_shape), mybir.dt.np(alloc.dtype)))
        self.in_names, self.out_names, self.out_avals = in_names, out_names, out_avals
        n_params, n_outs = len(in_names), len(out_avals)
        all_names = in_names + out_names
        if partition_name is not None:
            all_names.append(partition_name)

        def _body(*args):
            operands = list(args)
            if partition_name is not None:
                operands.append(partition_id_tensor())
            return tuple(_bass_exec_p.bind(
                *operands, out_avals=tuple(out_avals), in_names=tuple(all_names),
                out_names=tuple(out_names), lowering_input_output_aliases=(),
                sim_require_finite=False, sim_require_nnan=False, nc=nc))

        devices = jax.devices()[:n_cores]
        self.mesh = Mesh(np.asarray(devices), ("core",))
        self.sharding = NamedSharding(self.mesh, PartitionSpec("core"))
        in_specs = (PartitionSpec("core"),) * (n_params + n_outs)
        out_specs = (PartitionSpec("core"),) * n_outs
        self.fn = jax.jit(
            shard_map(_body, mesh=self.mesh, in_specs=in_specs,
                      out_specs=out_specs, check_rep=False),
            donate_argnums=tuple(range(n_params, n_params + n_outs)),
            keep_unused=True)
        zshapes = [(n_cores * a.shape[0], *a.shape[1:]) for a in out_avals]
        zdtypes = [a.dtype for a in out_avals]
        self.make_zeros = jax.jit(
            lambda: tuple(jax.numpy.zeros(sh, dt)
                          for sh, dt in zip(zshapes, zdtypes)),
            out_shardings=tuple(self.sharding for _ in zshapes))

    def prep(self, in_maps):
        return [self.jax.device_put(
            np.concatenate([np.asarray(in_maps[c][n]) for c in range(self.n_cores)],
                           axis=0), self.sharding)
            for n in self.in_names]

    def run(self, dev_in):
        return self.fn(*dev_in, *self.make_zeros())

    def split(self, outs):
        res = []
        for c in range(self.n_cores):
            res.append({n: np.asarray(outs[i]).reshape(
                self.n_cores, *self.out_avals[i].shape)[c]
                for i, n in enumerate(self.out_names)})
        return res


def get_runner():
    global _RUNNER
    if _RUNNER is None:
        _RUNNER = _Runner(build(), N_CORES)
    return _RUNNER


def kernel(x, freqs_cos, freqs_sin, mask, Wq, Wk, Wv, Wo):
    x = np.asarray(x, dtype=np.float32)
    in_maps = make_in_maps(np.asarray(x), np.asarray(freqs_cos),
                           np.asarray(freqs_sin), np.asarray(Wq),
                           np.asarray(Wk), np.asarray(Wv), np.asarray(Wo))
    r = get_runner()
    outs = r.run(r.prep(in_maps))
    res = r.split(outs)
    out = np.empty((B, S, HID), dtype=np.float32)
    for b in range(B):
        out[b] = res[2 * b]["y"] + res[2 * b + 1]["y"]
    return out
